# revision 8
# baseline (speedup 1.0000x reference)
"""Trainium2 Bass kernel for GroupedQuerySelfAttention (v5: K/V dedup via
split AllGathers + indirect-DMA slot permutation + head-outer rounds).

Problem: B=2, N=2048, D=2048, H=8 kv-heads, G=4 (32 query heads), C=64.

Sharding: 8 cores = 2 batches x 4 query-chunks of 512 rows. Each core
projects K/V only for ITS 512 seq rows (the old kernel duplicated the
full K/V projection 4x), then the 4-core batch group exchanges chunks
via four AllGather collectives sized/ordered so each lands just before
its first consumer on the serial collective device:
  KgA (K h0-3) ~[27, 68], VgA (V h0-3) ~[68, 110],
  KgB (K h4-7) ~[110, 151], VgB (V h4-7) ~[151, 193].
The gathered chunks land in LOCAL-slot order (slot 0 = own chunk, slot
1+j = global chunk (qc+1+j)%4) via indirect DMAs whose per-partition row
indices (idx[p,j] = ((qc+1+j)%4)*128 + p) are per-core INPUT DATA -- the
one rank-dependent step, expressed as data so the SPMD program stays
identical across cores.  Attention is invariant to the s-permutation;
denominators sum over all slots.  Dummy pool-queue DMAs after each
indirect group absorb the scheduler's reader-threshold over-counting.

Round 0 (slot 0) merges with the Q projection: Q chains run 2 j-blocks
wide (2 psum banks) so the QK stream keeps a 2-buf psum pool, and r0
head-groups interleave INTO the chain windows from a ready-backlog
(heads release at Q-chain HALF granularity -- h0-3 of group jc only
need jc's first two j-blocks), keeping the ACT exp stream fed through
the Q phases.  The remaining rounds run HEAD-OUTER (for each kv-head:
slots 1-3), so pair denominators finalize evenly through the window and
the O evacuations (zero lag) plus the 4-pass output projection (gated
on evacuated pairs, weights prefetched mid-pass) drip into the rounds'
PE slack.

Layouts (matmul inputs bf16, psum f32): xT [p, db, 512] local slice
(DMA'd in wkv_k/xt-interleaved 4-db pieces, alternating queues, with
warmup-filler transposes holding the p-state clock); KTA/KTB
[p, slot, kvb*512+s] slot-major K^T per head-half (separate tiles keep
indirect-write deps decoupled); VstA/VstB [p, slot, nb, h%4, 65] with
the ones column riding through the gather; QT g-major; S^T/E^T/PV with
q-partition PV output and denominators from the ones column.

Engine budget at 368.1us (cost model): PE 312us busy (84%), ACT 267
(exp stream [50, 344] ~92% fed), DVE 128, collectives 166 (hidden).
"""

import numpy as np
from contextlib import ExitStack

import concourse.bass as bass
import concourse.tile as tile
from concourse import bacc, mybir
from concourse.bass_utils import run_bass_kernel_spmd
from concourse.masks import make_identity

P = 128
B, N, D = 2, 2048, 2048
H, G, C = 8, 4, 64
HG = H * G                     # 32 query heads
NQ = 512                       # query rows per core
DB = D // P                    # 16 d-blocks
QB = NQ // P                   # 4 query blocks
CH = N // NQ                   # 4 seq chunks (slots)
SCALE = float(1.0 / np.sqrt(HG))
WARMUP = 24
FILL = 4
OFF_N = 0
KEEP = 6
RDIV = 4
PLAG = 2
LATE2 = 0
ELAG = 0
EPB = 10
F32 = mybir.dt.float32
BF16 = mybir.dt.bfloat16
I32 = mybir.dt.int32
AF = mybir.ActivationFunctionType
GROUPS = [[0, 1, 2, 3], [4, 5, 6, 7]]
VROW = 4 * 4 * (C + 1)         # 1040: one (half, slot) V region per partition


def build_program(n_cores=8, dbg=False):
    nc = bacc.Bacc("TRN2", target_bir_lowering=False, debug=False,
                   num_devices=n_cores)
    groups = GROUPS if n_cores == 8 else [[0]]
    gath = 4 if n_cores == 8 else 1
    dbg_t = {}
    if dbg:
        for nm, shp in [("dQT", [P, DB, NQ]), ("dKT", [P, CH, 4 * NQ]),
                        ("dVst", [P, 2, CH, 4, 4, C + 1]),
                        ("dOT", [P, DB, NQ])]:
            dbg_t[nm] = nc.dram_tensor(nm, shp, BF16, kind="ExternalOutput").ap()
        dbg_t["dOacc"] = nc.dram_tensor(
            "dOacc", [P, QB, HG // 2, 2, C + 1], F32, kind="ExternalOutput").ap()
    xt = nc.dram_tensor("xt", [4, P, 4, NQ], BF16, kind="ExternalInput").ap()
    wq = nc.dram_tensor("wq", [4, P, DB, NQ], BF16, kind="ExternalInput").ap()
    wkv = nc.dram_tensor("wkv", [P, DB, 2, NQ], BF16, kind="ExternalInput").ap()
    wp = nc.dram_tensor("wp", [P, DB, 4, NQ], BF16, kind="ExternalInput").ap()
    bp = nc.dram_tensor("bp", [D], F32, kind="ExternalInput").ap()
    idx = nc.dram_tensor("idx", [P, CH - 1], I32, kind="ExternalInput").ap()
    out = nc.dram_tensor("out", [QB, P, 4, NQ], BF16, kind="ExternalOutput").ap()

    with tile.TileContext(nc) as tc, ExitStack() as top:
        per = top.enter_context(tc.tile_pool(name="per", bufs=1))
        identb = per.tile([P, P], BF16, tag="identb")
        make_identity(nc, identb[:])
        ones = per.tile([P, 1], BF16, tag="ones")
        nc.gpsimd.memset(ones[:], 1.0)
        idxt = per.tile([P, CH - 1], I32, tag="idxt")
        nc.sync.dma_start(idxt[:], idx)
        Oacc = top.enter_context(tc.tile_pool(name="Oaccp", bufs=1)).tile(
            [P, QB, HG // 2, 2, C + 1], F32, tag="Oacc")
        def dram_tile(nm, shape):
            pool = top.enter_context(
                tc.tile_pool(name=f"dram_{nm}", bufs=1, space="DRAM"))
            return pool.tile(shape, BF16, tag=nm, name=nm)
        kinA = dram_tile("kinA", [P, 2 * NQ])
        koutA = dram_tile("koutA", [gath * P, 2 * NQ])
        kinB = dram_tile("kinB", [P, 2 * NQ])
        koutB = dram_tile("koutB", [gath * P, 2 * NQ])
        vinA = dram_tile("vinA", [P, VROW])
        voutA = dram_tile("voutA", [gath * P, VROW])
        vinB = dram_tile("vinB", [P, VROW])
        voutB = dram_tile("voutB", [gath * P, VROW])

        def allgather(dst, src):
            nc.gpsimd.collective_compute(
                "AllGather", mybir.AluOpType.bypass, replica_groups=groups,
                ins=[src[:].opt()], outs=[dst[:].opt()])

        dpad = top.enter_context(tc.tile_pool(name="dpad", bufs=1, space="DRAM"))
        padt = dpad.tile([1, 8], BF16, tag="padt", name="padt")
        padsb = per.tile([1, 8], BF16, tag="padsb")
        nc.gpsimd.memset(padsb[:], 0.0)

        def fetch_slots(dsts, src):
            """Indirect-fetch the 3 remote slots: dsts[j] (contiguous
            [P, rowlen] SBUF region) <- src row idx[p, j].  Trailing dummy
            DMAs keep the pool-queue completion counter ahead of readers
            whose thresholds over-count later queue items."""
            if n_cores != 8:
                return
            for j in range(CH - 1):
                nc.gpsimd.indirect_dma_start(
                    out=dsts[j], out_offset=None, in_=src[:],
                    in_offset=bass.IndirectOffsetOnAxis(
                        ap=idxt[:, j:j + 1], axis=0))
            for _ in range(6):
                nc.gpsimd.dma_start(padt[:], padsb[:])

        with ExitStack() as main:
            QT = main.enter_context(tc.tile_pool(name="QTp", bufs=1)).tile(
                [P, DB, NQ], BF16, tag="QT")
            KTA = main.enter_context(tc.tile_pool(name="KTAp", bufs=1)).tile(
                [P, CH, 2 * NQ], BF16, tag="KTA")
            KTB = main.enter_context(tc.tile_pool(name="KTBp", bufs=1)).tile(
                [P, CH, 2 * NQ], BF16, tag="KTB")
            VstA = main.enter_context(tc.tile_pool(name="VstAp", bufs=1)).tile(
                [P, CH, 4, 4, C + 1], BF16, tag="VstA")
            VstB = main.enter_context(tc.tile_pool(name="VstBp", bufs=1)).tile(
                [P, CH, 4, 4, C + 1], BF16, tag="VstB")
            for Vh in (VstA, VstB):
                nc.vector.tensor_copy(
                    Vh[:, 0, :, :, C:C + 1],
                    ones[:, None, None, :].to_broadcast((P, 4, 4, 1)))
            ep = main.enter_context(tc.tile_pool(name="ep", bufs=EPB))

            # ---------------- attention machinery ----------------
            class QkStream:
                def __init__(self, qkps, epool, w=2, etag="E", off_n=0,
                             stp=None, ebase=None):
                    self.qkps = qkps
                    self.ep = epool
                    self.W = w
                    self.etag = etag
                    self.tile = None
                    self.entries = []
                    self.slots = {}
                    self.off_n = off_n     # every off_n-th group -> Pool pow
                    self.stp = stp         # f32 sbuf staging pool
                    self.ebase = ebase     # [P,1] tile holding e**SCALE
                    self.ctr = 0

                def push(self, ch, h, g, sb4):
                    off = (h % 2) * C
                    if self.tile is None:
                        self.tile = self.qkps.tile([P, self.W, NQ], F32,
                                                   tag="qk")
                    slot = len(self.entries)
                    KT = KTA if h < 4 else KTB
                    kvb = (h // 2) % 2
                    nc.tensor.matmul(
                        self.tile[:, slot, :],
                        KT[off:off + C, ch,
                           kvb * NQ + sb4 * P:kvb * NQ + (sb4 + 1) * P],
                        QT[off:off + C, g * 4 + h // 2, :],
                        start=True, stop=True)
                    self.entries.append((ch, h, g, sb4))
                    if len(self.entries) == self.W:
                        self.flush()

                def flush(self):
                    if self.tile is None:
                        return
                    n = len(self.entries)
                    et = self.ep.tile([P, self.W, NQ], BF16, tag=self.etag)
                    self.ctr += 1
                    if self.off_n and self.ctr % self.off_n == 0:
                        # offload: DVE evacuates the psum, Pool computes
                        # (e**SCALE)**S == exp(S*SCALE) via pow
                        st = self.stp.tile([P, self.W, NQ], F32, tag="st")
                        nc.vector.tensor_copy(st[:, :n, :], self.tile[:, :n, :])
                        nc.gpsimd.tensor_tensor(
                            et[:, :n, :],
                            self.ebase[:, None, :].to_broadcast((P, n, NQ)),
                            st[:, :n, :], mybir.AluOpType.pow)
                    else:
                        nc.scalar.activation(et[:, :n, :], self.tile[:, :n, :],
                                             AF.Exp, scale=SCALE)
                    for i, key in enumerate(self.entries):
                        self.slots[key] = (et, i)
                    self.tile = None
                    self.entries = []

            def emit_qk_exp(ch, h, g, stream):
                for sb4 in range(4):
                    stream.push(ch, h, g, sb4)
                return stream

            def ensure_flushed(e):
                # flush only if this entry's exps haven't been emitted yet
                # (avoids splitting the steady-state W-groups)
                if any((e[0], e[1], e[2], sb4) not in e[3].slots
                       for sb4 in range(4)):
                    e[3].flush()

            def emit_pv(ch, h, g, stream, pvps):
                pv = pvps.tile([P, QB, P], F32, tag="pv")
                for qb in range(QB):
                    for sb4 in range(4):
                        et, slot = stream.slots[(ch, h, g, sb4)]
                        Vh = VstA if h < 4 else VstB
                        nc.tensor.matmul(
                            pv[:, qb, :C + 1],
                            et[:, slot, qb * P:(qb + 1) * P],
                            Vh[:, ch, sb4, h % 4, :],
                            start=(qb == 0 and sb4 == 0),
                            stop=(qb == QB - 1 and sb4 == 3))
                for sb4 in range(4):
                    del stream.slots[(ch, h, g, sb4)]
                pair, gp = h * 2 + g // 2, g % 2
                dst = Oacc[:, :, pair, gp, :]
                if ch == 0:
                    nc.vector.tensor_copy(dst, pv[:, :, :C + 1])
                else:
                    nc.vector.tensor_add(dst, dst, pv[:, :, :C + 1])

            def pop_pv(pend, pvps):
                e = pend.pop(0)
                ensure_flushed(e)
                emit_pv(*e, pvps)
                return (e[0], e[1] * G + e[2])

            def flush_pend(pend, pvps):
                flushed = []
                while pend:
                    flushed.append(pop_pv(pend, pvps))
                return flushed

            with ExitStack() as vscope:
                xts = vscope.enter_context(tc.tile_pool(name="xts", bufs=1))
                xT = xts.tile([P, DB, NQ], BF16, tag="xT")
                wkvp = vscope.enter_context(tc.tile_pool(name="wkvp", bufs=1))
                wkv_v = wkvp.tile([P, DB, NQ], BF16, tag="wkv_v")
                wqp = vscope.enter_context(tc.tile_pool(name="wqp", bufs=8))

                # ---- K local projection; KgA kicked asap ----
                with ExitStack() as s:
                    wkp = s.enter_context(tc.tile_pool(name="wkp", bufs=1))
                    wkv_k = wkp.tile([P, DB, NQ], BF16, tag="wkv_k")
                    # interleave wkv_k/xt in 4-db pieces so the db-outer
                    # K chains start at ~3us and stay fed; wkv_v queues
                    # behind the K-critical feed
                    for k4 in range(4):
                        sl = slice(4 * k4, 4 * k4 + 4)
                        enga = nc.sync if k4 % 2 == 0 else nc.scalar
                        engb = nc.scalar if k4 % 2 == 0 else nc.sync
                        enga.dma_start(wkv_k[:, sl, :], wkv[:, sl, 0, :])
                        engb.dma_start(xT[:, sl, :], xt[k4])
                    for hf in range(2):
                        nc.scalar.dma_start(wkv_v[:, hf * 8:(hf + 1) * 8, :],
                                            wkv[:, hf * 8:(hf + 1) * 8, 1, :])
                    # short PE warmup: busy streak from ~t=1us until the
                    # first K matmul's inputs land, ramping the p-state
                    wups = s.enter_context(
                        tc.tile_pool(name="wups", bufs=1, space="PSUM"))
                    wup = wups.tile([P, P], BF16, tag="wup")
                    for _ in range(WARMUP):
                        nc.tensor.matmul(wup[:], identb[:], identb[:],
                                         is_transpose=True,
                                         start=True, stop=True)
                    kps = s.enter_context(
                        tc.tile_pool(name="kps", bufs=1, space="PSUM"))
                    kp4 = kps.tile([P, 4, NQ], F32, tag="kp4")
                    for db in range(DB):
                        for jb in range(4):
                            nc.tensor.matmul(
                                kp4[:, jb, :],
                                wkv_k[:, db, jb * P:(jb + 1) * P],
                                xT[:, db, :],
                                start=(db == 0), stop=(db == DB - 1))
                        if db % 4 == 3 and db < DB - 1:
                            # filler transposes bridge the DMA-feed gap so
                            # the PE busy-streak (and p-state clock) holds
                            for _ in range(FILL):
                                nc.tensor.matmul(wup[:], identb[:], identb[:],
                                                 is_transpose=True,
                                                 start=True, stop=True)
                    for _ in range(30):
                        nc.tensor.matmul(wup[:], identb[:], identb[:],
                                         is_transpose=True,
                                         start=True, stop=True)
                    nc.vector.tensor_copy(KTA[:, 0, :], kp4[:, 0:2, :])
                    nc.sync.dma_start(kinA[:], KTA[:, 0, :])
                    allgather(koutA, kinA)
                    nc.vector.tensor_copy(KTB[:, 0, :], kp4[:, 2:4, :])
                    nc.scalar.dma_start(kinB[:], KTB[:, 0, :])

                # ---- V local projection; VgA/KgB/VgB queued ----
                with ExitStack() as vs0:
                    vps0 = vs0.enter_context(
                        tc.tile_pool(name="vps0", bufs=2, space="PSUM"))
                    for nb in range(4):
                        vp = vps0.tile([P, H, C], F32, tag="vp")
                        for db in range(DB):
                            nc.tensor.matmul(
                                vp[:], xT[:, db, nb * P:(nb + 1) * P],
                                wkv_v[:, db, :],
                                start=(db == 0), stop=(db == DB - 1))
                        nc.vector.tensor_copy(
                            VstA[:, 0, nb, :, :C], vp[:, 0:4, :])
                        nc.vector.tensor_copy(
                            VstB[:, 0, nb, :, :C], vp[:, 4:8, :])
                nc.scalar.dma_start(
                    vinA[:], VstA[:, 0].rearrange('p a b c -> p (a b c)'))
                nc.scalar.dma_start(
                    vinB[:], VstB[:, 0].rearrange('p a b c -> p (a b c)'))
                # each gather immediately followed by its indirect fetches,
                # so every fetch waits exactly its own producer
                fetch_slots([KTA[:, j + 1, :] for j in range(3)], koutA)
                allgather(voutA, vinA)
                fetch_slots([VstA[:, j + 1].rearrange('p a b c -> p (a b c)')
                             for j in range(3)], voutA)
                allgather(koutB, kinB)
                fetch_slots([KTB[:, j + 1, :] for j in range(3)], koutB)
                allgather(voutB, vinB)
                fetch_slots([VstB[:, j + 1].rearrange('p a b c -> p (a b c)')
                             for j in range(3)], voutB)

                # ---- merged Q projection + attention round 0 (slot 0) ----
                # Q chains run 2 j-blocks at a time (2 psum banks) so the
                # QK stream keeps a 2-buf psum pool; round-0 head-groups
                # interleave INTO the chain windows from a ready-backlog,
                # keeping the ACT exp stream fed through the Q phases.
                pvpsA = vscope.enter_context(
                    tc.tile_pool(name="pvpsA", bufs=1, space="PSUM"))
                r0ps = vscope.enter_context(ExitStack())
                qpool = r0ps.enter_context(
                    tc.tile_pool(name="qpool", bufs=1, space="PSUM"))
                spsA = r0ps.enter_context(
                    tc.tile_pool(name="spsA", bufs=2, space="PSUM"))
                pend = []
                streamA = QkStream(spsA, ep)
                ready = []

                def emit_ready(n):
                    for _ in range(n):
                        if not ready:
                            return
                        h, g = ready.pop(0)
                        pend.append((0, h, g, emit_qk_exp(0, h, g, streamA)))
                        if len(pend) > 2:
                            pop_pv(pend, pvpsA)

                for jc in range(4):
                    wts = []
                    for q4 in range(4):
                        wt = wqp.tile([P, 4, NQ], BF16, tag="wq")
                        nc.sync.dma_start(wt[:],
                                          wq[jc, :, q4 * 4:(q4 + 1) * 4, :])
                        wts.append(wt)
                    for half in range(2):
                        qp = qpool.tile([P, 2, NQ], F32, tag="qp")
                        for db in range(DB):
                            for j2 in range(2):
                                jb = half * 2 + j2
                                nc.tensor.matmul(
                                    qp[:, j2, :],
                                    wts[db // 4][:, db % 4,
                                                 jb * P:(jb + 1) * P],
                                    xT[:, db, :],
                                    start=(db == 0), stop=(db == DB - 1))
                            if db % RDIV == RDIV - 1:
                                emit_ready(1)
                        nc.vector.tensor_copy(
                            QT[:, jc * 4 + half * 2:jc * 4 + half * 2 + 2, :],
                            qp[:])
                        # heads h with h//2 in this half's j-blocks are now
                        # QT-complete for g=jc: release them to the backlog
                        ready += [(h, jc)
                                  for h in range(half * 4, half * 4 + 4)]
                    emit_ready(len(ready) - (KEEP if jc < 3 else 0))
                emit_ready(len(ready))
                # keep CARRY entries pending across the boundary so the
                # exp->PV pipeline stays warm through the pool transition
                while len(pend) > CARRY:
                    pop_pv(pend, pvpsA)
                streamA.flush()
                carry = pend
            # xT / wkv_k / wkv_v / wqp freed here

            # ---- head-outer rounds: for each kv-head, slots 1-3 ----
            # Pair denominators finalize evenly through the window, so the
            # O evacuations and the output-projection passes drip into the
            # whole rounds region instead of bunching at the end.
            OT = main.enter_context(tc.tile_pool(name="OTp", bufs=1)).tile(
                [P, DB, NQ], BF16, tag="OT")
            rp = main.enter_context(tc.tile_pool(name="rp", bufs=1))
            rec = rp.tile([P, QB, HG // 2, 2], F32, tag="rec")
            otp = main.enter_context(tc.tile_pool(name="otp", bufs=3))
            r3 = main.enter_context(ExitStack())
            rsc = r3.enter_context(ExitStack())
            qkpsB = rsc.enter_context(
                tc.tile_pool(name="qkpsB", bufs=2, space="PSUM"))
            pvpsB = rsc.enter_context(
                tc.tile_pool(name="pvpsB", bufs=1, space="PSUM"))
            trps = rsc.enter_context(
                tc.tile_pool(name="trps", bufs=1, space="PSUM"))
            opsA = r3.enter_context(
                tc.tile_pool(name="opsA", bufs=2, space="PSUM"))
            opshold = [opsA]

            def emit_evac(pair):
                nc.vector.reciprocal(rec[:, :, pair, :],
                                     Oacc[:, :, pair, :, C])
                trp = trps.tile([P, 2 * QB, P], BF16, tag="trp")
                for qb in range(QB):
                    ot = otp.tile([P, 2, C], BF16, tag="ot")
                    nc.vector.tensor_mul(
                        ot[:], Oacc[:, qb, pair, :, :C],
                        rec[:, qb, pair, :, None].to_broadcast((P, 2, C)))
                    nc.tensor.matmul(trp[:, qb, :], ot[:], identb[:],
                                     is_transpose=True,
                                     start=(qb == 0), stop=(qb == QB - 1))
                nc.vector.tensor_copy(OT[:, pair, :], trp[:, :QB, :])

            bpb = main.enter_context(tc.tile_pool(name="bpbp", bufs=1)).tile(
                [P, D], F32, tag="bpb")
            nc.sync.dma_start(bpb[:], bp[None, :].to_broadcast((P, D)))
            PASS_JB = [(0, 4), (4, 8), (8, 12), (12, 16)]
            wpqp = main.enter_context(tc.tile_pool(name="wpqp", bufs=8))
            wtq = {0: []}
            for ob in range(4):
                w0 = PASS_JB[0][1] - PASS_JB[0][0]
                wt = wpqp.tile([P, w0, NQ], BF16, tag="wpq")
                nc.sync.dma_start(wt[:], wp[:, PASS_JB[0][0]:PASS_JB[0][1], ob, :])
                wtq[0].append(wt)
            partp = main.enter_context(tc.tile_pool(name="partp", bufs=16))
            osbp = main.enter_context(tc.tile_pool(name="osbp", bufs=3))
            part = {}
            qstate = [0, 0]          # evacs done, pieces emitted

            def emit_piece():
                if qstate[1] >= 16 * len(PASS_JB):
                    return False
                k, rem = divmod(qstate[1], 16)
                jlo, jhi = PASS_JB[k]
                if qstate[0] < jhi:     # pass k needs pairs 0..jhi-1 evac'd
                    return False
                if rem == 8 and k < len(PASS_JB) - 1:
                    wtq[k + 1] = []
                    lo, hi = PASS_JB[k + 1]
                    for ob in range(4):
                        wt = wpqp.tile([P, hi - lo, NQ], BF16, tag="wpq")
                        nc.sync.dma_start(wt[:], wp[:, lo:hi, ob, :])
                        wtq[k + 1].append(wt)
                ob, qb = divmod(rem, 4)
                opA = opshold[0].tile([P, NQ], F32, tag="opA")
                for j4 in range(jhi - jlo):
                    nc.tensor.matmul(
                        opA[:], OT[:, jlo + j4, qb * P:(qb + 1) * P],
                        wtq[k][ob][:, j4, :],
                        start=(j4 == 0), stop=(j4 == jhi - jlo - 1))
                if k == 0:
                    pt = partp.tile([P, NQ], BF16, tag="part",
                                    name=f"part{ob}_{qb}")
                    nc.vector.tensor_add(pt[:], opA[:],
                                         bpb[:, ob * NQ:(ob + 1) * NQ])
                    part[(ob, qb)] = pt
                elif k < len(PASS_JB) - 1:
                    pt = part[(ob, qb)]
                    nc.vector.tensor_add(pt[:], pt[:], opA[:])
                else:
                    osb = osbp.tile([P, NQ], BF16, tag="osb")
                    nc.vector.tensor_add(osb[:], opA[:], part[(ob, qb)][:])
                    nc.sync.dma_start(out[qb, :, ob, :], osb[:])
                qstate[1] += 1
                return True

            evacq = []

            def queue_evac(done, lag):
                if done is not None and done % 2 == 1:
                    evacq.append((done // G) * 2 + (done % G) // 2)
                while len(evacq) > lag:
                    emit_evac(evacq.pop(0))
                    qstate[0] += 1

            pend3 = carry
            streamB = QkStream(qkpsB, ep)
            items = [(ch, h, g) for h in range(H)
                     for ch in range(1, CH) for g in range(G)]
            for ch, h, g in items:
                pend3.append((ch, h, g, emit_qk_exp(ch, h, g, streamB)))
                done = None
                if len(pend3) > PLAG:
                    dch, dhg = pop_pv(pend3, pvpsB)
                    if dch == CH - 1:
                        done = dhg
                queue_evac(done, ELAG)
                emit_piece()
                if LATE2 and (ch, h, g) >= (1, 6, 0):
                    emit_piece()
            for dch, dhg in flush_pend(pend3, pvpsB):
                queue_evac(dhg if dch == CH - 1 else None, 1)
            queue_evac(None, 0)
            while qstate[1] < 16 * len(PASS_JB):
                emit_piece()
            r3.close()

            if dbg:
                nc.sync.dma_start(dbg_t["dQT"][:], QT[:])
                nc.sync.dma_start(dbg_t["dOacc"][:], Oacc[:])
                nc.sync.dma_start(dbg_t["dOT"][:], OT[:])


    nc.compile()
    return nc


_nc_cache = None


def _prep_inputs(x, Wq, Wkv, Wp, bp):
    """Host-side layout prep (bf16 casts, transposes, reshapes)."""
    import ml_dtypes
    bf16 = ml_dtypes.bfloat16
    x = np.asarray(x, dtype=np.float32)
    Wq = (np.asarray(Wq, dtype=np.float32)
          .reshape(D, H, G, C).transpose(0, 2, 1, 3).reshape(D, D))
    wq_p = np.ascontiguousarray(
        Wq.reshape(DB, P, 4, NQ).transpose(2, 1, 0, 3)).astype(bf16)
    wkv_p = np.ascontiguousarray(
        np.asarray(Wkv, dtype=np.float32)
        .reshape(DB, P, 2, NQ).transpose(1, 0, 2, 3)).astype(bf16)
    wp_p = np.ascontiguousarray(
        np.asarray(Wp, dtype=np.float32)
        .reshape(DB, P, 4, NQ).transpose(1, 0, 2, 3)).astype(bf16)
    bp_p = np.ascontiguousarray(np.asarray(bp, dtype=np.float32))
    xts = []
    for b in range(B):
        for qc in range(CH):
            xs = x[b, qc * NQ:(qc + 1) * NQ, :]
            xts.append(np.ascontiguousarray(
                xs.T.reshape(4, 4, P, NQ).transpose(0, 2, 1, 3)).astype(bf16))
    return xts, wq_p, wkv_p, wp_p, bp_p


def make_in_maps(x, Wq, Wkv, Wp, bp):
    xts, wq_p, wkv_p, wp_p, bp_p = _prep_inputs(x, Wq, Wkv, Wp, bp)
    in_maps = []
    for c in range(8):
        qc = c % 4
        idx = np.stack([
            np.array([((qc + 1 + j) % 4) * P + p for j in range(CH - 1)],
                     np.int32)
            for p in range(P)])
        in_maps.append({
            "xt": xts[c],
            "wq": wq_p, "wkv": wkv_p, "wp": wp_p, "bp": bp_p,
            "idx": idx,
        })
    return in_maps


def kernel(x, Wq, Wkv, Wp, bp):
    global _nc_cache
    if _nc_cache is None:
        _nc_cache = build_program()
    nc = _nc_cache
    in_maps = make_in_maps(x, Wq, Wkv, Wp, bp)
    res = run_bass_kernel_spmd(nc, in_maps, list(range(8)))
    outp = np.empty((B, N, D), np.float32)
    for c in range(8):
        b, qc = c // 4, c % 4
        o = np.asarray(res.results[c]["out"], dtype=np.float32)
        outp[b, qc * NQ:(qc + 1) * NQ] = o.reshape(NQ, D)
    return outp
